# revision 23
# baseline (speedup 1.0000x reference)
"""BaiChuan attention layer on 8 Trainium2 NeuronCores.

Sharding: tensor-parallel over heads within groups of 4 cores (W_pack
column-parallel, o_proj column-parallel after AllGather of attention
outputs), data-parallel over the batch across the two groups.

Per-core dataflow (core c: batch b=c//4, rank r=c%4, heads 8r..8r+8):
  stage A: fused QKV projection with q,k in fp8e4m3 DoubleRow matmuls
           (2x PE) and v in bf16; all outputs stay in SBUF.  RoPE is
           fused into the PSUM->SBUF path per 512-token slice (ACT
           descale copy, SBUF->SBUF DMA half swap, DVE mul/mul/add with
           fp8 output).  v is transposed to natural layout on the PE.
           All weight tensors are pre-tiled on the host so every DMA is
           a single contiguous run per partition.
  stage B: per head causal attention, scores^T = k-blocks @ q (fp8),
           exp on ACT, causal mask on DVE, denominator via per-block
           ones-matmuls accumulated in PSUM, reciprocal_approx_fast,
           PV in bf16.  Two bf16 AllGathers (heads 0-3 after head 3,
           heads 4-7 after head 7) instead of one per head - they are
           latency-bound (~50us each).
  stage C: o_proj in bf16, split into G1 (heads 0-3, executes while the
           second AllGather is in flight) and G2 (heads 4-7 + G1
           staging add).
"""
import sys
sys.path.insert(0, '/opt/trn_rl_repo')
import numpy as np
import ml_dtypes

import concourse.bass as bass
from concourse import bacc
import concourse.mybir as mybir
from concourse.tile import TileContext
from concourse.bass_utils import run_bass_kernel_spmd
from concourse.masks import make_identity

f32 = mybir.dt.float32
f32r = mybir.dt.float32r
bf16 = mybir.dt.bfloat16
f8 = mybir.dt.float8e4
AF = mybir.ActivationFunctionType
DR = mybir.MatmulPerfMode.DoubleRow

B, S, H, NH = 2, 2048, 4096, 32
HD = H // NH                    # 128
THETA = 10000.0
NCORES, TPN = 8, 4              # 2 groups of 4 (DP over batch x TP over heads)
HPC = NH // TPN                 # 8 heads per core
JC = HPC * HD                   # 1024 per-core q (=k=v) width
SCALE = HD ** -0.5
GROUPS = [[0, 1, 2, 3], [4, 5, 6, 7]]
TB = 1024                       # stage-A token block
NTB = S // TB
NIB = H // 128                  # 32 contraction blocks
NG = S // 512                   # 4 query blocks per head
NKB = S // 128                  # 16 key blocks per head

SQ = 512.0                      # input/weight fp8 quant scale
SR = 256.0                      # roped q,k fp8 quant scale
RAW_SCALE = SR / (SQ * SQ)      # 2^-10, PSUM -> bf16 descale for q,k
ESC = SCALE / (SR * SR)         # exp() scale folding the fp8 scales


def build_nc():
    nc = bacc.Bacc(None)
    # pre-tiled host layouts: every slice is contiguous per partition
    hs8 = nc.declare_dram_parameter("hs8", [NTB, 128, NIB, TB], f8,
                                    isOutput=False)
    hsb = nc.declare_dram_parameter("hsb", [NTB, 128, NIB, TB], bf16,
                                    isOutput=False)
    w8 = nc.declare_dram_parameter("w8", [HPC, 128, NIB, 2 * HD], f8,
                                   isOutput=False)
    wvb = nc.declare_dram_parameter("wvb", [HPC, 128, NIB, HD], bf16,
                                    isOutput=False)
    wob = nc.declare_dram_parameter("wob", [2, 128, 16, JC], bf16,
                                    isOutput=False)
    cosf = nc.declare_dram_parameter("cosf", [HD, S], bf16, isOutput=False)
    sinm = nc.declare_dram_parameter("sinm", [HD, S], bf16, isOutput=False)
    masks = nc.declare_dram_parameter("masks", [4, 128, 512], bf16,
                                      isOutput=False)
    out = nc.declare_dram_parameter("out", [S, JC], f32, isOutput=True)

    # per-half attention outputs and their AllGathers (rank-major rows)
    attn_h = [nc.dram_tensor(f"attn_h{x}", [4, HD, S], bf16)
              for x in range(2)]
    ag = [nc.dram_tensor(f"ag{x}", [TPN * 4 * HD, S], bf16)
          for x in range(2)]
    ag_v = [t[:].rearrange("(j p) t -> p j t", p=128) for t in ag]

    with TileContext(nc) as tc:
        with tc.tile_pool(name="pers", bufs=1) as pp:
            ident = pp.tile([128, 128], bf16, tag="ident", bufs=1)
            make_identity(nc, ident[:])
            ones_b = pp.tile([128, 1], bf16, tag="ones_b", bufs=1)
            nc.vector.memset(ones_b[:], 1.0)
            # per-core q,k (roped, fp8) and v (natural, bf16); this pool
            # closes after stage B so stage C reuses the space
            pq_cm = tc.tile_pool(name="pq", bufs=1)
            pq = pq_cm.__enter__()
            rq8 = pq.tile([128, HPC, S], f8, tag="rq8", bufs=1)
            rk8 = pq.tile([128, HPC, S], f8, tag="rk8", bufs=1)
            v_sb = pq.tile([128, NKB, JC], bf16, tag="vsb", bufs=1)
            # hsb lives beside stKQ so its load overlaps the K/Q phase
            ph_cm = tc.tile_pool(name="phsb", bufs=1)
            ph = ph_cm.__enter__()

            hsb_t0 = ph.tile([128, NIB, TB], bf16, tag="hsb", bufs=1,
                             name="hsb_0")
            for d in range(4):
                nc.sync.dma_start(
                    out=hsb_t0[:, 8 * d:8 * (d + 1), :],
                    in_=hsb[:][0, :, 8 * d:8 * (d + 1), :])

            # ------------- K/Q projection (fp8 DoubleRow) + fused RoPE -----
            with nc.named_scope("stageKQ"), \
                 tc.tile_pool(name="stKQ", bufs=1) as pa, \
                 tc.tile_pool(name="psKQ", bufs=1, space="PSUM") as psA:
                cos_sb = pa.tile([128, S], bf16, tag="cos", bufs=1)
                sin_sb = pa.tile([128, S], bf16, tag="sin", bufs=1)
                for tb in range(NTB):
                    hs8_t = pa.tile([128, NIB, TB], f8, tag="hs8", bufs=1,
                                    name=f"hs8_{tb}")
                    for d in range(4):
                        nc.sync.dma_start(
                            out=hs8_t[:, 8 * d:8 * (d + 1), :],
                            in_=hs8[:][tb, :, 8 * d:8 * (d + 1), :])
                    if tb == 0:
                        nc.sync.dma_start(out=cos_sb[:], in_=cosf[:])
                        nc.sync.dma_start(out=sin_sb[:], in_=sinm[:])
                    for h in range(HPC):
                        w8_t = pa.tile([128, NIB, 2 * HD], f8, tag="w8",
                                       bufs=2, name=f"w8_{tb}_{h}")
                        nc.sync.dma_start(out=w8_t[:], in_=w8[:][h])
                        for dst, jo, tag in ((rk8, HD, "k"), (rq8, 0, "q")):
                            for tq in range(TB // 512):
                                ps = psA.tile([128, 512], f32, tag="psqk",
                                              bufs=3,
                                              name=f"ps_{tb}_{h}{tag}_{tq}")
                                for dd in range(NIB // 2):
                                    nc.tensor.matmul(
                                        ps[:],
                                        w8_t[:, 2 * dd:2 * dd + 2,
                                             jo:jo + HD],
                                        hs8_t[:, 2 * dd:2 * dd + 2,
                                              tq * 512:(tq + 1) * 512],
                                        start=(dd == 0),
                                        stop=(dd == NIB // 2 - 1),
                                        perf_mode=DR)
                                tcol = tb * TB + tq * 512
                                raw = pa.tile([128, 512], bf16, tag="raw",
                                              bufs=2,
                                              name=f"raw_{tb}_{h}{tag}_{tq}")
                                nc.scalar.activation(raw[:], ps[:], AF.Copy,
                                                     scale=RAW_SCALE)
                                sw = pa.tile([128, 512], bf16, tag="sw",
                                             bufs=2,
                                             name=f"sw_{tb}_{h}{tag}_{tq}")
                                nc.sync.dma_start(out=sw[0:64, :],
                                                  in_=raw[64:128, :])
                                nc.sync.dma_start(out=sw[64:128, :],
                                                  in_=raw[0:64, :])
                                t1 = pa.tile([128, 512], bf16, tag="rt",
                                             bufs=2,
                                             name=f"t1_{tb}_{h}{tag}_{tq}")
                                t2 = pa.tile([128, 512], bf16, tag="rt",
                                             bufs=2,
                                             name=f"t2_{tb}_{h}{tag}_{tq}")
                                with tc.high_priority():
                                    nc.vector.tensor_mul(
                                        t1[:], raw[:],
                                        cos_sb[:, tcol:tcol + 512])
                                    nc.vector.tensor_mul(
                                        t2[:], sw[:],
                                        sin_sb[:, tcol:tcol + 512])
                                    nc.vector.tensor_add(
                                        dst[:, h, tcol:tcol + 512],
                                        t1[:], t2[:])

            # ------------- V projection (bf16) + stage B interleaved -------
            def emit_v(pv, psV, hsb_t, tb, h):
                wv_t = pv.tile([128, NIB, HD], bf16, tag="wv",
                               bufs=2, name=f"wv_{tb}_{h}")
                nc.sync.dma_start(out=wv_t[:], in_=wvb[:][h])
                for tq in range(TB // 512):
                    psv = psV.tile([128, 512], f32, tag="psv",
                                   bufs=1, name=f"psv_{tb}_{h}_{tq}")
                    for i in range(NIB):
                        nc.tensor.matmul(
                            psv[:], wv_t[:, i, :],
                            hsb_t[:, i, tq * 512:(tq + 1) * 512],
                            start=(i == 0), stop=(i == NIB - 1))
                    vst = pv.tile([128, 512], bf16, tag="vst",
                                  bufs=1, name=f"vst_{tb}_{h}_{tq}")
                    nc.scalar.copy(vst[:], psv[:])
                    kb0 = tb * (TB // 128) + tq * 4
                    for q4 in range(4):
                        pst = psV.tile([128, 128], bf16, tag="pst",
                                       bufs=2,
                                       name=f"pst_{tb}_{h}_{tq}_{q4}")
                        nc.tensor.transpose(
                            pst[:],
                            vst[:, q4 * 128:(q4 + 1) * 128],
                            ident[:])
                        nc.scalar.copy(
                            v_sb[:, kb0 + q4, h * HD:(h + 1) * HD],
                            pst[:])

            def emit_b_head(pb, psB, mask_sb, h):
                with nc.named_scope(f"head{h}"):
                    attn = pb.tile([128, S], bf16, tag="attn",
                                   bufs=2, name=f"attn_{h}")
                    for g in range(NG):
                        nkb = 4 * g + 4
                        po = psB.tile([128, 512], f32, tag="po",
                                      bufs=2, name=f"po_{h}_{g}")
                        acc = pb.tile([128, 512], bf16, tag="acc",
                                      bufs=2, name=f"acc_{h}_{g}")
                        for kb in range(nkb):
                            # diagonal tiles only have causal content in
                            # columns >= 128*v; skip the rest
                            diag = kb >= 4 * g
                            v = kb - 4 * g
                            x0 = 128 * v if diag else 0
                            qsl = slice(g * 512 + x0, (g + 1) * 512)
                            csl = slice(x0, 512)
                            pss = psB.tile([128, 512], f32,
                                           tag="pss", bufs=3,
                                           name=f"pss_{h}_{g}_{kb}")
                            nc.tensor.matmul(
                                pss[:, csl],
                                rk8[:, h, kb * 128:(kb + 1) * 128],
                                rq8[:, h, qsl],
                                start=True, stop=True)
                            pt = pb.tile([128, 512], bf16, tag="pt",
                                         bufs=4,
                                         name=f"pt_{h}_{g}_{kb}")
                            if diag or kb % 3 != 1:
                                nc.scalar.activation(
                                    pt[:, csl], pss[:, csl], AF.Exp,
                                    scale=ESC)
                            else:
                                # offload psum read to DVE so the exp
                                # runs in ACT 2x mode
                                sc = pb.tile([128, 512], bf16,
                                             tag="sc", bufs=3,
                                             name=f"sc_{h}_{g}_{kb}")
                                nc.vector.tensor_copy(sc[:], pss[:])
                                nc.scalar.activation(
                                    pt[:], sc[:], AF.Exp, scale=ESC)
                            if diag:
                                nc.vector.tensor_mul(
                                    pt[:, csl], pt[:, csl],
                                    mask_sb[:, v, csl])
                            if kb == 0:
                                nc.vector.tensor_copy(acc[:], pt[:])
                            else:
                                nc.vector.tensor_add(acc[:, csl],
                                                     acc[:, csl],
                                                     pt[:, csl])
                            nc.tensor.matmul(
                                po[:, csl],
                                v_sb[:, kb, h * HD:(h + 1) * HD],
                                pt[:, csl],
                                start=(kb == 0),
                                stop=(kb == nkb - 1),
                                skip_group_check=True)
                        pden = psB.tile([128, 512], f32, tag="pss",
                                        bufs=3, name=f"pden_{h}_{g}")
                        nc.tensor.matmul(pden[0:1, :], ones_b[:], acc[:],
                                         start=True, stop=True)
                        den1 = pb.tile([1, 512], f32, tag="den1",
                                       bufs=2, name=f"den1_{h}_{g}")
                        nc.scalar.copy(den1[:], pden[0:1, :])
                        rd1 = pb.tile([1, 512], f32, tag="rd1",
                                      bufs=2, name=f"rd1_{h}_{g}")
                        nc.vector.reciprocal_approx_fast(rd1[:], den1[:])
                        rden = pb.tile([128, 512], f32, tag="rden",
                                       bufs=2, name=f"rden_{h}_{g}")
                        nc.gpsimd.partition_broadcast(rden[:], rd1[:])
                        nc.vector.tensor_mul(
                            attn[:, g * 512:(g + 1) * 512], po[:],
                            rden[:])
                    x = h // 4
                    nc.sync.dma_start(out=attn_h[x][:][h % 4],
                                      in_=attn[:])
                    if h % 4 == 3:
                        nc.gpsimd.collective_compute(
                            "AllGather", mybir.AluOpType.bypass,
                            replica_groups=GROUPS,
                            ins=[attn_h[x][:]], outs=[ag[x][:]])

            with nc.named_scope("stageVB"), \
                 tc.tile_pool(name="stV", bufs=1) as pv, \
                 tc.tile_pool(name="psV", bufs=1, space="PSUM") as psV:
                for h in range(HPC):
                    emit_v(pv, psV, hsb_t0, 0, h)
                hsb_t1 = ph.tile([128, NIB, TB], bf16, tag="hsb", bufs=1,
                                 name="hsb_1")
                for d in range(4):
                    nc.sync.dma_start(
                        out=hsb_t1[:, 8 * d:8 * (d + 1), :],
                        in_=hsb[:][1, :, 8 * d:8 * (d + 1), :])
                with nc.named_scope("stageB"), \
                     tc.tile_pool(name="stB", bufs=1) as pb, \
                     tc.tile_pool(name="psB", bufs=1, space="PSUM") as psB:
                    mask_sb = pb.tile([128, 4, 512], bf16, tag="mask",
                                      bufs=1)
                    nc.sync.dma_start(out=mask_sb[:],
                                      in_=masks[:].rearrange("v p x -> p v x"))
                    for h in range(HPC):
                        emit_v(pv, psV, hsb_t1, 1, h)
                        emit_b_head(pb, psB, mask_sb, h)

            # close the hsb and q/k/v pools so stage C reuses the space
            ph_cm.__exit__(None, None, None)
            pq_cm.__exit__(None, None, None)

            # ---------------- stage C: o_proj G1 + G2 ----------------------
            with nc.named_scope("stageC"), \
                 tc.tile_pool(name="stC", bufs=1) as pc, \
                 tc.tile_pool(name="psC", bufs=1, space="PSUM") as psC:
                wo1 = pc.tile([128, 16, JC], bf16, tag="wo", bufs=1,
                              name="wo1")
                for d in range(4):
                    nc.sync.dma_start(out=wo1[:, 4 * d:4 * (d + 1), :],
                                      in_=wob[:][0, :, 4 * d:4 * (d + 1), :])
                g1s = pc.tile([128, 16, JC], bf16, tag="g1s", bufs=1)
                at1 = pc.tile([128, 16, S], bf16, tag="at", bufs=1,
                              name="at1")
                for d in range(4):
                    nc.sync.dma_start(out=at1[:, 4 * d:4 * (d + 1), :],
                                      in_=ag_v[0][:, 4 * d:4 * (d + 1), :])
                for tb in range(NKB):
                    pscs = [psC.tile([128, 512], f32, tag="psc",
                                     bufs=4, name=f"psc1_{tb}_{mc}")
                            for mc in range(2)]
                    for jb in range(16):
                        for mc in range(2):
                            nc.tensor.matmul(
                                pscs[mc][:],
                                at1[:, jb, tb * 128:(tb + 1) * 128],
                                wo1[:, jb, mc * 512:(mc + 1) * 512],
                                start=(jb == 0), stop=(jb == 15))
                    for mc in range(2):
                        nc.scalar.copy(
                            g1s[:, tb, mc * 512:(mc + 1) * 512],
                            pscs[mc][:])
                # G2: heads 4-7 (needs AG2); wo2/at2 reuse the G1 buffers
                wo2 = pc.tile([128, 16, JC], bf16, tag="wo", bufs=1,
                              name="wo2")
                for d in range(4):
                    nc.sync.dma_start(out=wo2[:, 4 * d:4 * (d + 1), :],
                                      in_=wob[:][1, :, 4 * d:4 * (d + 1), :])
                at2 = pc.tile([128, 16, S], bf16, tag="at2", bufs=1,
                              name="at2")
                for d in range(4):
                    nc.sync.dma_start(out=at2[:, 4 * d:4 * (d + 1), :],
                                      in_=ag_v[1][:, 4 * d:4 * (d + 1), :])
                for tb in range(NKB):
                    ps2s = [psC.tile([128, 512], f32, tag="psc",
                                     bufs=4, name=f"psc2_{tb}_{mc}")
                            for mc in range(2)]
                    for jb in range(16):
                        for mc in range(2):
                            nc.tensor.matmul(
                                ps2s[mc][:],
                                at2[:, jb, tb * 128:(tb + 1) * 128],
                                wo2[:, jb, mc * 512:(mc + 1) * 512],
                                start=(jb == 0), stop=(jb == 15))
                    for mc in range(2):
                        oc = pc.tile([128, 512], f32, tag="oc",
                                     bufs=2, name=f"oc_{tb}_{mc}")
                        nc.vector.tensor_add(
                            oc[:], ps2s[mc][:],
                            g1s[:, tb, mc * 512:(mc + 1) * 512])
                        nc.sync.dma_start(
                            out=out[:][tb * 128:(tb + 1) * 128,
                                       mc * 512:(mc + 1) * 512],
                            in_=oc[:])

    nc.finalize()
    return nc


_NC_CACHE = None


def _get_nc():
    global _NC_CACHE
    if _NC_CACHE is None:
        _NC_CACHE = build_nc()
    return _NC_CACHE


def _host_inputs(hidden_states, positions, w_pack, w_o):
    hidden_states = np.asarray(hidden_states, dtype=np.float32)
    positions = np.asarray(positions)
    w_pack = np.asarray(w_pack, dtype=np.float32)
    w_o = np.asarray(w_o, dtype=np.float32)

    half = HD // 2
    inv_freq = (1.0 / (THETA ** (np.arange(half, dtype=np.float32) / half)))

    # causal mask variants for the 4 diagonal (128x512) tiles of a q-block
    masks = np.empty((4, 128, 512), dtype=np.float32)
    xs = np.arange(512)[None, :]
    ps = np.arange(128)[:, None]
    for v in range(4):
        masks[v] = (xs >= ps + 128 * v).astype(np.float32)

    def to_f8(x):
        return np.clip(x, -240.0, 240.0).astype(ml_dtypes.float8_e4m3)

    def tile_pnj(wT):
        # [H, W] -> [128 p, NIB n, W] pre-tiled (p-major, contiguous)
        return np.ascontiguousarray(
            wT.reshape(NIB, 128, -1).transpose(1, 0, 2))

    in_maps = []
    for c in range(NCORES):
        b, r = divmod(c, TPN)
        heads = np.arange(HPC * r, HPC * (r + 1))
        rows = (heads[:, None] * HD + np.arange(HD)[None, :]).reshape(-1)
        # w8[h]: [128, NIB, 2*HD] with q cols then k cols of head h
        wq = w_pack[rows] * SQ                                   # [JC, H]
        wk = w_pack[H + rows] * SQ
        w8 = np.empty((HPC, 128, NIB, 2 * HD), dtype=np.float32)
        wv = np.empty((HPC, 128, NIB, HD), dtype=np.float32)
        wvs = w_pack[2 * H + rows]
        for h in range(HPC):
            w8[h, :, :, :HD] = tile_pnj(
                np.ascontiguousarray(wq[h * HD:(h + 1) * HD].T))
            w8[h, :, :, HD:] = tile_pnj(
                np.ascontiguousarray(wk[h * HD:(h + 1) * HD].T))
            wv[h] = tile_pnj(
                np.ascontiguousarray(wvs[h * HD:(h + 1) * HD].T))
        # o_proj m-shard; gathered rows are (rank, head-in-half, d)
        wo_shard = w_o[JC * r:JC * (r + 1), :]                   # [JC, H]
        woT_full = np.ascontiguousarray(wo_shard.T)              # [H=j, JC]
        arr = woT_full.reshape(TPN, HPC, HD, JC)                 # [r',g_h,d]
        wob = np.empty((2, 128, 16, JC), dtype=np.float32)
        for x in range(2):
            half_rows = np.ascontiguousarray(
                arr[:, 4 * x:4 * (x + 1)]).reshape(16, HD, JC)
            # [16 jb, 128 d, JC] -> [128 p=d, 16 jb, JC]
            wob[x] = half_rows.transpose(1, 0, 2)
        hsT = np.ascontiguousarray(hidden_states[b].T)           # [H, S]
        hst = tile_pnj(hsT).reshape(128, NIB, NTB, TB) \
                           .transpose(2, 0, 1, 3)                # [tb,p,n,TB]
        ang = positions[b].astype(np.float32)[None, :] * inv_freq[:, None]
        cos_t = np.cos(ang).astype(np.float32)                   # [64, S]
        sin_t = np.sin(ang).astype(np.float32)
        cosf = np.concatenate([cos_t, cos_t], axis=0)            # [128, S]
        sinm = np.concatenate([-sin_t, sin_t], axis=0)
        in_maps.append({
            "hs8": to_f8(np.ascontiguousarray(hst) * SQ),
            "hsb": np.ascontiguousarray(hst).astype(ml_dtypes.bfloat16),
            "w8": to_f8(w8),
            "wvb": wv.astype(ml_dtypes.bfloat16),
            "wob": wob.astype(ml_dtypes.bfloat16),
            "cosf": cosf.astype(ml_dtypes.bfloat16),
            "sinm": sinm.astype(ml_dtypes.bfloat16),
            "masks": masks.astype(ml_dtypes.bfloat16),
        })
    return in_maps


def kernel(hidden_states, positions, w_pack, w_o):
    import os
    os.environ["BASS_NEVER_TRACE"] = "1"
    nc = _get_nc()
    in_maps = _host_inputs(hidden_states, positions, w_pack, w_o)
    res = run_bass_kernel_spmd(nc, in_maps, list(range(NCORES)))
    out = np.empty((B, S, H), dtype=np.float32)
    for c in range(NCORES):
        b, r = divmod(c, TPN)
        out[b][:, JC * r:JC * (r + 1)] = res.results[c]["out"]
    return out
